# revision 1
# baseline (speedup 1.0000x reference)
"""CrossLayerTranscoder with global batch-wise top-k masking on 8 TRN2 cores.

  pre = x @ W_enc + b_enc          (relu skipped: tau > 0 makes it a no-op)
  out = pre * (pre >= tau)         tau = k-th largest of relu(pre) globally

Sharding: dict-dim (W columns) across 8 cores; each core computes its
[4096, 2048] slice.  The global top-k threshold tau is found exactly via a
distributed candidate reduction: each core extracts the top-16 of every
(row, 512-col) chunk on-device (2 rounds of max8/match_replace).  With
k/chunks ~ Poisson(2) hits per chunk, 16 >= all hits a.s., so the k-th
largest of the candidate union equals the global k-th largest.  The host
merges the small candidate sets, computes tau, and a second pass masks.
"""

import sys
import types

import numpy as np

P = 128
N_TOTAL = 4096
K_DIM = 768
DICT = 16384
N_CORES = 8
DICT_SH = DICT // N_CORES
NFREE = 512
KCH = K_DIM // P
M_TILES = N_TOTAL // P
N_CHUNKS = DICT_SH // NFREE
NEG = -1e30

_cache = {}


def _build_pass_a(cand_rounds=2):
    import concourse.mybir as mybir
    import concourse.tile as tile
    from concourse import bacc

    nc = bacc.Bacc("TRN2", target_bir_lowering=False, debug=False,
                   num_devices=N_CORES)
    f32 = mybir.dt.float32
    xT = nc.dram_tensor("xT", [K_DIM, N_TOTAL], f32, kind="ExternalInput")
    w = nc.dram_tensor("w", [K_DIM, DICT_SH], f32, kind="ExternalInput")
    b = nc.dram_tensor("b", [1, DICT_SH], f32, kind="ExternalInput")
    pre = nc.dram_tensor("pre", [N_TOTAL, DICT_SH], f32, kind="ExternalOutput")
    ncand = 8 * cand_rounds
    cand = nc.dram_tensor("cand", [N_TOTAL, ncand * N_CHUNKS], f32,
                          kind="ExternalOutput")

    with tile.TileContext(nc) as tc:
        with (
            tc.tile_pool(name="resident", bufs=1) as rpool,
            tc.tile_pool(name="work", bufs=4) as wpool,
            tc.tile_pool(name="mr", bufs=3) as mrpool,
            tc.tile_pool(name="cands", bufs=4) as cpool,
            tc.tile_pool(name="psum", bufs=6, space="PSUM") as psum_pool,
            tc.tile_pool(name="bias_psum", bufs=1, space="PSUM") as bps_pool,
        ):
            w_sb = rpool.tile([P, KCH, DICT_SH], f32)
            nc.sync.dma_start(w_sb[:], w.ap().rearrange("(c p) n -> p c n", p=P))
            b_sb = rpool.tile([1, DICT_SH], f32)
            nc.sync.dma_start(b_sb[:], b.ap())
            ones_sb = rpool.tile([1, P], f32)
            nc.vector.memset(ones_sb[:], 1.0)

            bias_sb = rpool.tile([P, N_CHUNKS, NFREE], f32)
            for n in range(N_CHUNKS):
                bps = bps_pool.tile([P, NFREE], f32)
                nc.tensor.matmul(bps[:], ones_sb[:],
                                 b_sb[:, n * NFREE:(n + 1) * NFREE],
                                 start=True, stop=True)
                nc.scalar.copy(bias_sb[:, n, :], bps[:])

            xT_sb = rpool.tile([P, M_TILES, KCH, P], f32)
            xT_r = xT.ap().rearrange("(c p) (mt mc) -> p mt c mc", p=P, mc=P)
            for m in range(M_TILES):
                nc.sync.dma_start(xT_sb[:, m], xT_r[:, m])

            for m in range(M_TILES):
                for n in range(N_CHUNKS):
                    ps = psum_pool.tile([P, NFREE], f32)
                    for k in range(KCH):
                        nc.tensor.matmul(ps[:], xT_sb[:, m, k, :],
                                         w_sb[:, k, n * NFREE:(n + 1) * NFREE],
                                         start=(k == 0), stop=(k == KCH - 1))
                    sb = wpool.tile([P, NFREE], f32, tag="sb")
                    nc.vector.tensor_add(sb[:], ps[:], bias_sb[:, n, :])
                    nc.sync.dma_start(
                        pre.ap()[m * P:(m + 1) * P, n * NFREE:(n + 1) * NFREE],
                        sb[:])
                    ct = cpool.tile([P, ncand], f32, tag="ct")
                    src = sb
                    for r in range(cand_rounds):
                        nc.vector.max(ct[:, r * 8:(r + 1) * 8], src[:])
                        if r + 1 < cand_rounds:
                            mr = mrpool.tile([P, NFREE], f32, tag="mr")
                            nc.vector.match_replace(
                                mr[:], ct[:, r * 8:(r + 1) * 8], src[:], NEG)
                            src = mr
                    nc.sync.dma_start(
                        cand.ap()[m * P:(m + 1) * P, n * ncand:(n + 1) * ncand],
                        ct[:])
    nc.compile()
    return nc


def _build_pass_b():
    import concourse.mybir as mybir
    import concourse.tile as tile
    from concourse import bacc

    nc = bacc.Bacc("TRN2", target_bir_lowering=False, debug=False,
                   num_devices=N_CORES)
    f32 = mybir.dt.float32
    pre = nc.dram_tensor("pre", [N_TOTAL, DICT_SH], f32, kind="ExternalInput")
    tau = nc.dram_tensor("tau", [P, 1], f32, kind="ExternalInput")
    out = nc.dram_tensor("out", [N_TOTAL, DICT_SH], f32, kind="ExternalOutput")

    FD = 2048
    with tile.TileContext(nc) as tc:
        with (
            tc.tile_pool(name="taup", bufs=1) as tpool,
            tc.tile_pool(name="work", bufs=4) as wpool,
            tc.tile_pool(name="maskp", bufs=4) as mpool,
        ):
            taub = tpool.tile([P, 1], f32)
            nc.sync.dma_start(taub[:], tau.ap())
            pre_r = pre.ap().rearrange("(mt p) n -> mt p n", p=P)
            out_r = out.ap().rearrange("(mt p) n -> mt p n", p=P)
            for m in range(M_TILES):
                for j in range(DICT_SH // FD):
                    t = wpool.tile([P, FD], f32, tag="t")
                    nc.sync.dma_start(t[:], pre_r[m, :, j * FD:(j + 1) * FD])
                    msk = mpool.tile([P, FD], f32, tag="m")
                    nc.vector.tensor_scalar(msk[:], t[:], taub[:], None,
                                            op0=mybir.AluOpType.is_ge)
                    nc.vector.tensor_mul(t[:], t[:], msk[:])
                    nc.sync.dma_start(out_r[m, :, j * FD:(j + 1) * FD], t[:])
    nc.compile()
    return nc


def _get_kernels():
    if "a" not in _cache:
        _cache["a"] = _build_pass_a()
        _cache["b"] = _build_pass_b()
    return _cache["a"], _cache["b"]


def kernel(x, W_enc, b_enc, top_k):
    from concourse.bass_utils import run_bass_kernel_spmd

    x = np.asarray(x, np.float32)
    W_enc = np.asarray(W_enc, np.float32)
    b_enc = np.asarray(b_enc, np.float32)
    top_k = int(top_k)
    k_tot = top_k * x.shape[0]

    nc_a, nc_b = _get_kernels()

    xT = np.ascontiguousarray(x.T)
    ins_a = []
    for c in range(N_CORES):
        sl = slice(c * DICT_SH, (c + 1) * DICT_SH)
        ins_a.append({
            "xT": xT,
            "w": np.ascontiguousarray(W_enc[:, sl]),
            "b": np.ascontiguousarray(b_enc[sl]).reshape(1, -1),
        })
    res_a = run_bass_kernel_spmd(nc_a, ins_a, core_ids=list(range(N_CORES)))

    # host-side global merge: exact k-th largest of the candidate union.
    cands = np.concatenate(
        [res_a.results[c]["cand"].ravel() for c in range(N_CORES)])
    # relu semantics: only positive values are ever kept; if k exceeds the
    # number of positive candidates, every positive value is kept (tau -> 0+).
    if k_tot <= cands.size:
        tau = np.partition(cands, -k_tot)[-k_tot]
    else:
        tau = np.float32(0.0)
    tau = max(tau, np.float32(np.finfo(np.float32).tiny))

    taut = np.full((P, 1), tau, np.float32)
    ins_b = [{"pre": res_a.results[c]["pre"], "tau": taut}
             for c in range(N_CORES)]
    res_b = run_bass_kernel_spmd(nc_b, ins_b, core_ids=list(range(N_CORES)))

    out = np.concatenate([res_b.results[c]["out"] for c in range(N_CORES)],
                         axis=1)
    return out.astype(np.float32)


# revision 2
# speedup vs baseline: 1.6244x; 1.6244x over previous
"""CrossLayerTranscoder with global batch-wise top-k masking on 8 TRN2 cores.

Reference computation:
    pre = relu(x @ W_enc + b_enc)            [4096, 16384]
    keep the global top-(top_k * 4096) entries, zero the rest.

Device algorithm (single pass, dict-sharded over 8 cores):
  * GEMM in split-f32r precision: x and W are split into hi (11 mantissa
    bits) + lo parts; hi*hi + hi*lo + lo*hi at float32r full rate gives
    fp32-level accuracy at 3 cycles/row instead of fp32's 4.
  * Transposed orientation (partition = dict col, free = rows) so the bias
    is per-partition and fuses into the ACT relu that drains PSUM.
  * Distributed top-k: each core extracts, for every (dict col, 128-row
    sub-chunk), the top-8 values + indices (DVE max8 / max_index).  With
    k/(#sub-chunks) ~ Poisson(0.5) kept elements per sub-chunk, top-8
    covers every globally-kept element a.s.
  * Global merge on host: tau = k-th largest of the candidate union (equal
    to the global k-th largest), then scatter the >= tau candidates into
    the zero output (count-exact at ties, lowest flat index first, matching
    jax.lax.top_k).
"""

import numpy as np

P = 128
N_TOTAL = 4096
K_DIM = 768
DICT = 16384
N_CORES = 8
DICT_SH = DICT // N_CORES     # 2048
KCH = K_DIM // P              # 6
R_BLK = 512
R_BLOCKS = N_TOTAL // R_BLK   # 8
D_TILES = DICT_SH // P        # 16
SPLIT_BITS = 12               # low mantissa bits dropped in the hi part

_cache = {}


def _build_sparse(sub):
    import concourse.mybir as mybir
    import concourse.tile as tile
    from concourse import bacc

    f32 = mybir.dt.float32
    f32r = mybir.dt.float32r
    u32 = mybir.dt.uint32
    NSUB = R_BLK // sub
    CW = 8 * NSUB

    nc = bacc.Bacc("TRN2", target_bir_lowering=False, debug=False,
                   num_devices=N_CORES)
    xh = nc.dram_tensor("xh", [K_DIM, N_TOTAL], f32r, kind="ExternalInput")
    xl = nc.dram_tensor("xl", [K_DIM, N_TOTAL], f32r, kind="ExternalInput")
    wh = nc.dram_tensor("wh", [K_DIM, DICT_SH], f32r, kind="ExternalInput")
    wl = nc.dram_tensor("wl", [K_DIM, DICT_SH], f32r, kind="ExternalInput")
    b = nc.dram_tensor("b", [P, D_TILES], f32, kind="ExternalInput")
    cval = nc.dram_tensor("cval", [DICT_SH, R_BLOCKS * CW], f32,
                          kind="ExternalOutput")
    cidx = nc.dram_tensor("cidx", [DICT_SH, R_BLOCKS * CW], u32,
                          kind="ExternalOutput")

    with tile.TileContext(nc) as tc:
        with (
            tc.tile_pool(name="resident", bufs=1) as rpool,
            tc.tile_pool(name="xstream", bufs=2) as xpool,
            tc.tile_pool(name="work", bufs=6) as wpool,
            tc.tile_pool(name="cand", bufs=3) as cpool,
            tc.tile_pool(name="psum", bufs=6, space="PSUM") as psum_pool,
        ):
            wh_sb = rpool.tile([P, KCH, DICT_SH], f32r)
            wl_sb = rpool.tile([P, KCH, DICT_SH], f32r)
            b_sb = rpool.tile([P, D_TILES], f32)
            nc.sync.dma_start(b_sb[:], b.ap())

            xh_r = xh.ap().rearrange("(c p) (rb rr) -> p rb c rr", p=P, rr=R_BLK)
            xl_r = xl.ap().rearrange("(c p) (rb rr) -> p rb c rr", p=P, rr=R_BLK)
            wh_r = wh.ap().rearrange("(c p) n -> p c n", p=P)
            wl_r = wl.ap().rearrange("(c p) n -> p c n", p=P)

            # first x block before W so the PE critical path is short; W in
            # d-chunks so early d-tiles can start before the full load lands
            xh_t0 = xpool.tile([P, KCH, R_BLK], f32r, tag="xh")
            xl_t0 = xpool.tile([P, KCH, R_BLK], f32r, tag="xl")
            for k in range(KCH):
                nc.sync.dma_start(xh_t0[:, k], xh_r[:, 0, k])
                nc.sync.dma_start(xl_t0[:, k], xl_r[:, 0, k])
            # ramped W chunking: fine at the start, coarse after
            edges = [0, 128, 256, 512, 1024, 1536, 2048]
            for q0, q1 in zip(edges[:-1], edges[1:]):
                qs = slice(q0, q1)
                for k in range(KCH):
                    nc.sync.dma_start(wh_sb[:, k, qs], wh_r[:, k, qs])
                    nc.sync.dma_start(wl_sb[:, k, qs], wl_r[:, k, qs])

            for r in range(R_BLOCKS):
                if r == 0:
                    xh_t, xl_t = xh_t0, xl_t0
                else:
                    xh_t = xpool.tile([P, KCH, R_BLK], f32r, tag="xh")
                    xl_t = xpool.tile([P, KCH, R_BLK], f32r, tag="xl")
                    for k in range(KCH):
                        nc.sync.dma_start(xh_t[:, k], xh_r[:, r, k])
                        nc.sync.dma_start(xl_t[:, k], xl_r[:, r, k])
                for d in range(D_TILES):
                    ps = psum_pool.tile([P, R_BLK], mybir.dt.float32)
                    dsl = slice(d * P, (d + 1) * P)
                    i = 0
                    for k in range(KCH):
                        for wt, xt in ((wh_sb, xh_t), (wh_sb, xl_t),
                                       (wl_sb, xh_t)):
                            nc.tensor.matmul(
                                ps[:], wt[:, k, dsl], xt[:, k, :],
                                start=(i == 0), stop=(i == 3 * KCH - 1))
                            i += 1
                    sb = wpool.tile([P, R_BLK], f32, tag="sb")
                    nc.scalar.activation(sb[:], ps[:],
                                         mybir.ActivationFunctionType.Relu,
                                         bias=b_sb[:, d:d + 1], scale=1.0)
                    cv = cpool.tile([P, CW], f32, tag="cv")
                    ci = cpool.tile([P, CW], u32, tag="ci")
                    for s in range(NSUB):
                        sl = slice(s * sub, (s + 1) * sub)
                        cs = slice(s * 8, (s + 1) * 8)
                        nc.vector.max(cv[:, cs], sb[:, sl])
                        nc.vector.max_index(ci[:, cs], cv[:, cs], sb[:, sl])
                    nc.sync.dma_start(cval.ap()[dsl, r * CW:(r + 1) * CW], cv[:])
                    nc.sync.dma_start(cidx.ap()[dsl, r * CW:(r + 1) * CW], ci[:])
    nc.compile()
    return nc


def _get_kernel(sub):
    if sub not in _cache:
        _cache[sub] = _build_sparse(sub)
    return _cache[sub]


def _split(a, bits=SPLIT_BITS):
    a = np.ascontiguousarray(a, np.float32)
    hi = (a.view(np.uint32)
          & np.uint32((0xFFFFFFFF << bits) & 0xFFFFFFFF)).view(np.float32)
    return hi, (a - hi).astype(np.float32)


def kernel(x, W_enc, b_enc, top_k):
    from concourse.bass_utils import run_bass_kernel_spmd

    x = np.ascontiguousarray(np.asarray(x), np.float32)
    W_enc = np.ascontiguousarray(np.asarray(W_enc), np.float32)
    b_enc = np.ascontiguousarray(np.asarray(b_enc), np.float32).ravel()
    top_k = int(np.asarray(top_k))
    k_tot = top_k * x.shape[0]
    out = np.zeros((N_TOTAL, DICT), np.float32)
    if k_tot <= 0:
        return out

    # sub-chunk size: expected kept per sub-chunk is top_k * sub / DICT;
    # top-8 per sub-chunk needs that (plus tail) well under 8.
    if top_k <= 96:
        sub = 128
    elif top_k <= 192:
        sub = 64
    else:
        sub = 32
    NSUB = R_BLK // sub
    CW = 8 * NSUB

    nc = _get_kernel(sub)

    xT = np.ascontiguousarray(x.T)
    xh, xl = _split(xT)
    ins = []
    for c in range(N_CORES):
        sl = slice(c * DICT_SH, (c + 1) * DICT_SH)
        wsh = np.ascontiguousarray(W_enc[:, sl])
        whi, wlo = _split(wsh)
        bsh = np.ascontiguousarray(b_enc[sl]).reshape(D_TILES, P).T.copy()
        ins.append({"xh": xh, "xl": xl, "wh": whi, "wl": wlo, "b": bsh})

    res = run_bass_kernel_spmd(nc, ins, core_ids=list(range(N_CORES)))

    # ---- global merge (host): exact tau + count-exact scatter ----
    vals = np.stack([res.results[c]["cval"] for c in range(N_CORES)])
    idxs = np.stack([res.results[c]["cidx"] for c in range(N_CORES)])
    flat = vals.ravel()
    k_eff = min(k_tot, flat.size)
    tau = np.partition(flat, -k_eff)[-k_eff]

    sel = flat >= tau
    fidx = np.flatnonzero(sel)
    v = flat[fidx]
    ii = idxs.ravel()[fidx].astype(np.int64)
    c_, rem = np.divmod(fidx, DICT_SH * R_BLOCKS * CW)
    dcol, rem = np.divmod(rem, R_BLOCKS * CW)
    rb, rem = np.divmod(rem, CW)
    s, _ = np.divmod(rem, 8)
    row = rb * R_BLK + s * sub + ii
    col = c_ * DICT_SH + dcol

    if tau > 0:
        n_gt = int((v > tau).sum())
        need = k_tot - n_gt
        at_tau = np.flatnonzero(v == tau)
        if at_tau.size > need:
            # count-exact tie-break: keep lowest flat index, like lax.top_k
            order = np.argsort(row[at_tau] * DICT + col[at_tau], kind="stable")
            drop = at_tau[order[need:]]
            keep = np.ones(v.size, bool)
            keep[drop] = False
            v, row, col = v[keep], row[keep], col[keep]
    else:
        # k exceeds the positive count: only positive values are visible
        # (setting zeros at zero positions is a no-op)
        keep = v > 0
        v, row, col = v[keep], row[keep], col[keep]

    out[row, col] = v
    return out


# revision 6
# speedup vs baseline: 1.6533x; 1.0178x over previous
"""CrossLayerTranscoder with global batch-wise top-k masking on 8 TRN2 cores.

Reference computation:
    pre = relu(x @ W_enc + b_enc)            [4096, 16384]
    keep the global top-(top_k * 4096) entries, zero the rest.

Device algorithm (single pass, dict-sharded over 8 cores):
  * GEMM in split-f32r precision: x and W are split into hi (11 mantissa
    bits) + lo parts; hi*hi + hi*lo + lo*hi at float32r full rate gives
    fp32-level accuracy at 3 cycles/row instead of fp32's 4.
  * Transposed orientation (partition = dict col, free = rows) so the bias
    is per-partition and fuses into the ACT relu that drains PSUM.
  * Distributed top-k: each core extracts, for every (dict col, 128-row
    sub-chunk), the top-8 values + indices (DVE max8 / max_index).  With
    k/(#sub-chunks) ~ Poisson(0.5) kept elements per sub-chunk, top-8
    covers every globally-kept element a.s.
  * Global merge on host: tau = k-th largest of the candidate union (equal
    to the global k-th largest), then scatter the >= tau candidates into
    the zero output (count-exact at ties, lowest flat index first, matching
    jax.lax.top_k).
"""

import numpy as np

P = 128
N_TOTAL = 4096
K_DIM = 768
DICT = 16384
N_CORES = 8
DICT_SH = DICT // N_CORES     # 2048
KCH = K_DIM // P              # 6
R_BLK = 512
R_BLOCKS = N_TOTAL // R_BLK   # 8
D_TILES = DICT_SH // P        # 16
SPLIT_BITS = 12               # low mantissa bits dropped in the hi part

_cache = {}


def _build_sparse(sub):
    import concourse.mybir as mybir
    import concourse.tile as tile
    from concourse import bacc

    f32 = mybir.dt.float32
    f32r = mybir.dt.float32r
    u32 = mybir.dt.uint32
    NSUB = R_BLK // sub
    CW = 8 * NSUB

    MASK = (0xFFFFFFFF << SPLIT_BITS) & 0xFFFFFFFF

    nc = bacc.Bacc("TRN2", target_bir_lowering=False, debug=False,
                   num_devices=N_CORES)
    xT = nc.dram_tensor("xT", [K_DIM, N_TOTAL], f32, kind="ExternalInput")
    wh = nc.dram_tensor("wh", [K_DIM, DICT_SH], f32r, kind="ExternalInput")
    wl = nc.dram_tensor("wl", [K_DIM, DICT_SH], f32r, kind="ExternalInput")
    b = nc.dram_tensor("b", [P, D_TILES], f32, kind="ExternalInput")
    cval = nc.dram_tensor("cval", [DICT_SH, R_BLOCKS * CW], f32,
                          kind="ExternalOutput")
    cidx = nc.dram_tensor("cidx", [DICT_SH, R_BLOCKS * CW], u32,
                          kind="ExternalOutput")

    with tile.TileContext(nc) as tc:
        with (
            tc.tile_pool(name="resident", bufs=1) as rpool,
            tc.tile_pool(name="xraw", bufs=6) as xrpool,
            tc.tile_pool(name="xstream", bufs=2) as xpool,
            tc.tile_pool(name="work", bufs=6) as wpool,
            tc.tile_pool(name="cand", bufs=3) as cpool,
            tc.tile_pool(name="psum", bufs=8, space="PSUM") as psum_pool,
        ):
            wh_sb = rpool.tile([P, KCH, DICT_SH], f32r)
            wl_sb = rpool.tile([P, KCH, DICT_SH], f32r)
            b_sb = rpool.tile([P, D_TILES], f32)
            nc.sync.dma_start(b_sb[:], b.ap())

            xT_r = xT.ap().rearrange("(c p) (rb rr) -> p rb c rr", p=P, rr=R_BLK)
            wh_r = wh.ap().rearrange("(c p) n -> p c n", p=P)
            wl_r = wl.ap().rearrange("(c p) n -> p c n", p=P)

            def load_split_x(r):
                """DMA one f32 r-block of x and split hi/lo on DVE.

                hi = f32r-rounded x (the copy's output rounding), lo = the
                residual, itself f32r-rounded on output; hi + lo carries
                ~22 mantissa bits into the 3-term matmul."""
                xh_t = xpool.tile([P, KCH, R_BLK], f32r, tag="xh")
                xl_t = xpool.tile([P, KCH, R_BLK], f32r, tag="xl")
                for k in range(KCH):
                    xf = xrpool.tile([P, R_BLK], f32, tag="xf")
                    nc.sync.dma_start(xf[:], xT_r[:, r, k])
                    nc.vector.tensor_copy(xh_t[:, k], xf[:])
                    nc.vector.tensor_sub(xl_t[:, k], xf[:],
                                         xh_t[:, k].bitcast(f32))
                return xh_t, xl_t

            # first x block before W so the PE critical path is short; W in
            # d-chunks so early d-tiles can start before the full load lands
            xh_t0, xl_t0 = load_split_x(0)
            # ramped W chunking: fine at the start, coarse after
            edges = [0, 128, 256, 512, 1024, 1536, 2048]
            for q0, q1 in zip(edges[:-1], edges[1:]):
                qs = slice(q0, q1)
                for k in range(KCH):
                    nc.sync.dma_start(wh_sb[:, k, qs], wh_r[:, k, qs])
                    nc.sync.dma_start(wl_sb[:, k, qs], wl_r[:, k, qs])

            for r in range(R_BLOCKS):
                if r == 0:
                    xh_t, xl_t = xh_t0, xl_t0
                else:
                    xh_t, xl_t = load_split_x(r)
                for d in range(D_TILES):
                    ps = psum_pool.tile([P, R_BLK], mybir.dt.float32)
                    dsl = slice(d * P, (d + 1) * P)
                    i = 0
                    for k in range(KCH):
                        for wt, xt in ((wh_sb, xh_t), (wh_sb, xl_t),
                                       (wl_sb, xh_t)):
                            nc.tensor.matmul(
                                ps[:], wt[:, k, dsl], xt[:, k, :],
                                start=(i == 0), stop=(i == 3 * KCH - 1))
                            i += 1
                    sb = wpool.tile([P, R_BLK], f32, tag="sb")
                    nc.scalar.activation(sb[:], ps[:],
                                         mybir.ActivationFunctionType.Relu,
                                         bias=b_sb[:, d:d + 1], scale=1.0)
                    cv = cpool.tile([P, CW], f32, tag="cv")
                    ci = cpool.tile([P, CW], u32, tag="ci")
                    for s in range(NSUB):
                        sl = slice(s * sub, (s + 1) * sub)
                        cs = slice(s * 8, (s + 1) * 8)
                        nc.vector.max(cv[:, cs], sb[:, sl])
                        nc.vector.max_index(ci[:, cs], cv[:, cs], sb[:, sl])
                    nc.sync.dma_start(cval.ap()[dsl, r * CW:(r + 1) * CW], cv[:])
                    nc.sync.dma_start(cidx.ap()[dsl, r * CW:(r + 1) * CW], ci[:])
    nc.compile()
    return nc


def _get_kernel(sub):
    if sub not in _cache:
        _cache[sub] = _build_sparse(sub)
    return _cache[sub]


def _split(a, bits=SPLIT_BITS):
    a = np.ascontiguousarray(a, np.float32)
    hi = (a.view(np.uint32)
          & np.uint32((0xFFFFFFFF << bits) & 0xFFFFFFFF)).view(np.float32)
    return hi, (a - hi).astype(np.float32)


def kernel(x, W_enc, b_enc, top_k):
    from concourse.bass_utils import run_bass_kernel_spmd

    x = np.ascontiguousarray(np.asarray(x), np.float32)
    W_enc = np.ascontiguousarray(np.asarray(W_enc), np.float32)
    b_enc = np.ascontiguousarray(np.asarray(b_enc), np.float32).ravel()
    top_k = int(np.asarray(top_k))
    k_tot = top_k * x.shape[0]
    out = np.zeros((N_TOTAL, DICT), np.float32)
    if k_tot <= 0:
        return out

    # sub-chunk size: expected kept per sub-chunk is top_k * sub / DICT;
    # top-8 per sub-chunk needs that (plus tail) well under 8.
    if top_k <= 96:
        sub = 128
    elif top_k <= 192:
        sub = 64
    else:
        sub = 32
    NSUB = R_BLK // sub
    CW = 8 * NSUB

    nc = _get_kernel(sub)

    xT = np.ascontiguousarray(x.T)
    ins = []
    for c in range(N_CORES):
        sl = slice(c * DICT_SH, (c + 1) * DICT_SH)
        wsh = np.ascontiguousarray(W_enc[:, sl])
        whi, wlo = _split(wsh)
        bsh = np.ascontiguousarray(b_enc[sl]).reshape(D_TILES, P).T.copy()
        ins.append({"xT": xT, "wh": whi, "wl": wlo, "b": bsh})

    res = run_bass_kernel_spmd(nc, ins, core_ids=list(range(N_CORES)))

    # ---- global merge (host): exact tau + count-exact scatter ----
    vals = np.stack([res.results[c]["cval"] for c in range(N_CORES)])
    idxs = np.stack([res.results[c]["cidx"] for c in range(N_CORES)])
    flat = vals.ravel()
    k_eff = min(k_tot, flat.size)
    tau = np.partition(flat, -k_eff)[-k_eff]

    sel = flat >= tau
    fidx = np.flatnonzero(sel)
    v = flat[fidx]
    ii = idxs.ravel()[fidx].astype(np.int64)
    c_, rem = np.divmod(fidx, DICT_SH * R_BLOCKS * CW)
    dcol, rem = np.divmod(rem, R_BLOCKS * CW)
    rb, rem = np.divmod(rem, CW)
    s, _ = np.divmod(rem, 8)
    row = rb * R_BLK + s * sub + ii
    col = c_ * DICT_SH + dcol

    if tau > 0:
        n_gt = int((v > tau).sum())
        need = k_tot - n_gt
        at_tau = np.flatnonzero(v == tau)
        if at_tau.size > need:
            # count-exact tie-break: keep lowest flat index, like lax.top_k
            order = np.argsort(row[at_tau] * DICT + col[at_tau], kind="stable")
            drop = at_tau[order[need:]]
            keep = np.ones(v.size, bool)
            keep[drop] = False
            v, row, col = v[keep], row[keep], col[keep]
    else:
        # k exceeds the positive count: only positive values are visible
        # (setting zeros at zero positions is a no-op)
        keep = v > 0
        v, row, col = v[keep], row[keep], col[keep]

    out[row, col] = v
    return out
